# revision 15
# baseline (speedup 1.0000x reference)
"""Baseline kernel + fp8-DR output projection graft (kernel_bplus).

Identical to the original 469794ns baseline except:
- Output projection runs fp8 DoubleRow: ot tiles are fp8 (scaled 1/256),
  wo is fp8 (scaled 16x, OS = WS^2 so all factors cancel), 2 matmuls/qs
  instead of 4.
- id2 comes from a host-provided identity via DMA (faster startup than
  make_identity).
"""

import math
import sys

import numpy as np

for _p in ("/opt/trn_rl_repo", "/opt/pypackages"):
    if _p not in sys.path:
        sys.path.append(_p)

import ml_dtypes

N = 16384
F = 512
MD = 128
P = 128
NCORES = 8
NQ = N // NCORES
QB = 512
NQB = NQ // QB
JT = 128
NJT = N // JT
FK = F // P
CH = 512
NCH = N // CH
GK = 16
NG = NJT // GK
WS = 16.0
OS = 256.0
SCALE = 1.0 / math.sqrt(MD) / (WS * WS)

_BF16 = ml_dtypes.bfloat16
_FP8 = ml_dtypes.float8_e4m3fn


def _build():
    import concourse.bass as bass  # noqa: F401
    import concourse.tile as tile
    from concourse import bacc, mybir

    f32 = mybir.dt.float32
    bf16 = mybir.dt.bfloat16
    fp8 = mybir.dt.float8e4
    DR = mybir.MatmulPerfMode.DoubleRow
    AF = mybir.ActivationFunctionType
    ALU = mybir.AluOpType

    nc = bacc.Bacc("TRN2", target_bir_lowering=False, debug=False,
                   num_devices=NCORES)

    xt = nc.declare_dram_parameter("xt", [F, N], fp8, isOutput=False)
    wq = nc.declare_dram_parameter("wq", [F, MD], fp8, isOutput=False)
    wk = nc.declare_dram_parameter("wk", [F, MD], fp8, isOutput=False)
    wv = nc.declare_dram_parameter("wv", [F, F], fp8, isOutput=False)
    wo = nc.declare_dram_parameter("wo", [F, F], fp8, isOutput=False)
    bq = nc.declare_dram_parameter("bq", [MD, 1], f32, isOutput=False)
    bo = nc.declare_dram_parameter("bo", [1, F], f32, isOutput=False)
    id_in = nc.declare_dram_parameter("id128", [P, P], fp8, isOutput=False)
    out = nc.declare_dram_parameter("out", [NQ, F], f32, isOutput=True)

    with tile.TileContext(nc) as tc:
        with (
            tc.tile_pool(name="persist", bufs=1) as pp,
            tc.tile_pool(name="stream", bufs=4) as sp,
            tc.tile_pool(name="work", bufs=3) as wkp,
            tc.tile_pool(name="pssc", bufs=3, space="PSUM") as ps_sc,
            tc.tile_pool(name="pso", bufs=4, space="PSUM") as ps_o,
            tc.tile_pool(name="psesum", bufs=1, space="PSUM") as ps_es,
        ):
            # ---- persistent constants ------------------------------------
            id2 = pp.tile([P, 2, P], fp8, tag="id2")
            nc.sync.dma_start(out=id2[:, 0, :], in_=id_in[:])
            nc.sync.dma_start(out=id2[:, 1, :], in_=id_in[:])
            wq_a = pp.tile([P, FK, MD], fp8, tag="wqa")
            wk_a = pp.tile([P, FK, MD], fp8, tag="wka")
            wv_a = pp.tile([P, FK, F], fp8, tag="wva")
            wo2 = [pp.tile([P, 2, F], fp8, tag=f"wo{u}", name=f"wo{u}")
                   for u in range(2)]
            for k in range(FK):
                nc.sync.dma_start(out=wk_a[:, k, :], in_=wk[k * P:(k + 1) * P, :])
                nc.sync.dma_start(out=wv_a[:, k, :], in_=wv[k * P:(k + 1) * P, :])
            for u in range(2):
                for h in range(2):
                    r = 2 * u + h
                    nc.scalar.dma_start(out=wo2[u][:, h, :],
                                        in_=wo[r * P:(r + 1) * P, :])
                nc.scalar.dma_start(out=wq_a[:, 2 * u, :],
                                    in_=wq[2 * u * P:(2 * u + 1) * P, :])
                nc.scalar.dma_start(out=wq_a[:, 2 * u + 1, :],
                                    in_=wq[(2 * u + 1) * P:(2 * u + 2) * P, :])
            bq_t = pp.tile([MD, 1], f32, tag="bq")
            nc.scalar.dma_start(out=bq_t[:], in_=bq[:])
            bo_r = pp.tile([P, F], f32, tag="bor")
            nc.scalar.dma_start(out=bo_r[:], in_=bo[:].to_broadcast((P, F)))
            ones_f = pp.tile([P, 1], bf16, tag="ones")
            nc.vector.memset(ones_f[:], 1.0)

            # ---- persistent activations -----------------------------------
            ktg = [pp.tile([P, GK * JT], fp8, tag=f"ktg{g}", name=f"ktg{g}")
                   for g in range(NG)]
            vg = [pp.tile([P, GK * F], fp8, tag=f"vg{g}", name=f"vg{g}")
                  for g in range(NG)]
            qt = pp.tile([P, NQ], fp8, tag="qt")

            # ---- PE warmup during the initial DMA wait (HAM un-throttle) --
            warm_ps = ps_sc.tile([P, P], f32, tag="sc", name="warm_ps")
            for wi in range(28):
                nc.tensor.matmul(warm_ps[:], id2[:, 0, :], id2[:, 0, :],
                                 start=(wi == 0), stop=(wi == 27))
            warm_s = pp.tile([P, P], bf16, tag="warms")
            nc.scalar.copy(warm_s[:], warm_ps[:])

            # ---- prologue: project Q^T, K^T, V (fp8 DoubleRow) ------------
            for ch in range(NCH):
                xtc = sp.tile([P, FK, CH], fp8, tag="xtc")
                xt4 = xt.rearrange("(k p) n -> p k n", p=P)
                dma_eng = nc.gpsimd if ch % 2 == 0 else nc.sync
                if ch < 2:
                    for k in range(FK):
                        dma_eng.dma_start(
                            out=xtc[:, k, :],
                            in_=xt[k * P:(k + 1) * P, ch * CH:(ch + 1) * CH])
                else:
                    dma_eng.dma_start(
                        out=xtc[:], in_=xt4[:, :, ch * CH:(ch + 1) * CH])
                g, off = ch // 4, (ch % 4) * CH
                pk = ps_es.tile([P, CH], f32, tag="esum", name="pk")
                for h in range(2):
                    nc.tensor.matmul(pk[:], wk_a[:, 2 * h:2 * h + 2, :],
                                     xtc[:, 2 * h:2 * h + 2, :],
                                     start=(h == 0), stop=(h == 1), perf_mode=DR)
                nc.scalar.copy(ktg[g][:, off:off + CH], pk[:])
                if ch < NQ // CH:
                    pq = ps_sc.tile([P, CH], f32, tag="sc", name="pq")
                    for h in range(2):
                        nc.tensor.matmul(pq[:], wq_a[:, 2 * h:2 * h + 2, :],
                                         xtc[:, 2 * h:2 * h + 2, :],
                                         start=(h == 0), stop=(h == 1),
                                         perf_mode=DR)
                    nc.scalar.activation(qt[:, ch * CH:(ch + 1) * CH], pq[:],
                                         AF.Identity, bias=bq_t[:], scale=1.0)
                for js in range(CH // JT):
                    jt_g = ch * (CH // JT) + js
                    voff = (jt_g % GK) * F
                    pv = ps_o.tile([P, F], f32, tag="oacc", name="pv")
                    for h in range(2):
                        nc.tensor.matmul(
                            pv[:], xtc[:, 2 * h:2 * h + 2, js * JT:(js + 1) * JT],
                            wv_a[:, 2 * h:2 * h + 2, :],
                            start=(h == 0), stop=(h == 1), perf_mode=DR)
                    if jt_g % 2 == 0:
                        nc.vector.tensor_copy(vg[jt_g // GK][:, voff:voff + F],
                                              pv[:])
                    else:
                        nc.scalar.copy(vg[jt_g // GK][:, voff:voff + F], pv[:])

            # ---- attention: flat pipeline over all (q-block, key-pair) ----
            NP2 = NJT // 2
            SUMS_PAT = {1: "D", 2: "D", 3: "G", 4: "D", 5: "D", 6: "D",
                        7: "G"}

            def scores(gjt):
                qbb, jt_i = gjt // NJT, gjt % NJT
                g, r = jt_i // GK, jt_i % GK
                psc = ps_sc.tile([P, QB], f32, tag="sc", name="psc")
                nc.tensor.matmul(psc[:], ktg[g][:, r * JT:(r + 1) * JT],
                                 qt[:, qbb * QB:(qbb + 1) * QB],
                                 start=True, stop=True)
                return psc

            pending = {j: scores(j) for j in range(3)}
            state = {}
            deferred = [None]

            def epilogue(st):
                ot2 = st["ot2"]
                recip_p = wkp.tile([P, QB // P], f32, tag="recipp", bufs=2,
                                   name="recip_p")
                s23 = st["s23"]
                for qs in range(QB // P):
                    pt = ps_sc.tile([P, 1], f32, tag="sc", name="pt")
                    nc.tensor.matmul(pt[:], s23[:, qs * P:(qs + 1) * P],
                                     ones_f[:], start=True, stop=True)
                    nc.vector.reciprocal(recip_p[:, qs:qs + 1], pt[:])
                    pf = ps_sc.tile([P, F], f32, tag="sc", name="pf")
                    for u in range(2):
                        nc.tensor.matmul(
                            pf[:], ot2[u][:, :, qs * P:(qs + 1) * P], wo2[u][:],
                            start=(u == 0), stop=(u == 1), perf_mode=DR)
                    out_t = wkp.tile([P, F], f32, tag="outt", bufs=2, name="out_t")
                    nc.vector.scalar_tensor_tensor(
                        out_t[:], pf[:], recip_p[:, qs:qs + 1], bo_r[:],
                        ALU.mult, ALU.add)
                    row0 = st["qb"] * QB + qs * P
                    nc.sync.dma_start(out=out[row0:row0 + P, :], in_=out_t[:])

            for gp_i in range(NQB * NP2):
                qb, p_i = gp_i // NP2, gp_i % NP2
                if p_i == 0:
                    state = {
                        "qb": qb,
                        "po": [ps_o.tile([P, QB], f32, tag="oacc", name="oacc")
                               for _ in range(FK)],
                        "esum": ps_es.tile([P, QB], f32, tag="esum",
                                           name="esum"),
                        "acc_d": wkp.tile([P, 2 * QB], bf16, tag="accd", bufs=2,
                                          name="acc_d"),
                        "acc_g": wkp.tile([P, 2 * QB], bf16, tag="accg", bufs=2,
                                          name="acc_g"),
                        "seen": {"d": False, "g": False},
                    }
                jt0 = 2 * p_i
                g, r0 = jt0 // GK, jt0 % GK
                etp = wkp.tile([P, 2 * QB], fp8, tag="et", bufs=6)
                for h in range(2):
                    psc = pending.pop(qb * NJT + jt0 + h)
                    nc.scalar.activation(etp[:, h * QB:(h + 1) * QB], psc[:],
                                         AF.Exp, scale=SCALE)
                    nxt = qb * NJT + jt0 + h + 3
                    if nxt < NQB * NJT:
                        pending[nxt] = scores(nxt)
                et3 = etp.rearrange("p (h q) -> p h q", h=2)
                if p_i % 8 == 0:
                    nc.tensor.matmul(state["esum"][:], id2[:], et3,
                                     start=(p_i == 0), stop=(p_i == NP2 - 8),
                                     perf_mode=DR)
                else:
                    kind = SUMS_PAT[p_i % 8]
                    eng, acc, key = ((nc.vector, state["acc_d"], "d")
                                     if kind == "D"
                                     else (nc.gpsimd, state["acc_g"], "g"))
                    if not state["seen"][key]:
                        eng.tensor_copy(acc[:], etp[:])
                        state["seen"][key] = True
                    else:
                        eng.tensor_tensor(acc[:], acc[:], etp[:], ALU.add)
                vg4 = vg[g].rearrange("p (t h f) -> p t h f", h=2, f=F)
                for ft in range(FK):
                    nc.tensor.matmul(
                        state["po"][ft][:],
                        vg4[:, r0 // 2, :, ft * P:(ft + 1) * P],
                        et3, start=(p_i == 0), stop=(p_i == NP2 - 1),
                        perf_mode=DR)
                if p_i == 1 and deferred[0] is not None:
                    epilogue(deferred[0])
                    deferred[0] = None
                if p_i == NP2 - 1:
                    ot2 = [wkp.tile([P, 2, QB], fp8, tag=f"ot{u}", bufs=2,
                                    name=f"ot2_{u}") for u in range(2)]
                    for u in range(2):
                        nc.scalar.activation(
                            ot2[u][:, 0, :], state["po"][2 * u][:],
                            AF.Copy, scale=1.0 / OS)
                        nc.vector.tensor_scalar_mul(
                            ot2[u][:, 1, :], state["po"][2 * u + 1][:],
                            1.0 / OS)
                    state["ot2"] = ot2
                    acc_d, acc_g = state["acc_d"], state["acc_g"]
                    nc.vector.tensor_tensor(acc_d[:], acc_d[:], acc_g[:],
                                            ALU.add)
                    s23 = wkp.tile([P, QB], bf16, tag="s23", bufs=2,
                                   name="s23")
                    nc.vector.tensor_tensor(s23[:], acc_d[:, 0:QB],
                                            acc_d[:, QB:2 * QB], ALU.add)
                    nc.vector.tensor_tensor(s23[:], s23[:], state["esum"][:],
                                            ALU.add)
                    state["s23"] = s23
                    deferred[0] = state
            epilogue(deferred[0])

    nc.compile()
    return nc


_CACHED = {}


def _get_nc():
    if "nc" not in _CACHED:
        _CACHED["nc"] = _build()
    return _CACHED["nc"]


def _make_in_maps(x, Wq, bq, Wk, bk, Wv, bv, Wo, bo):
    x = np.asarray(x, dtype=np.float32)
    xt_full = np.ascontiguousarray(x.T)                     # [F, N] f32
    wq_8 = (WS * np.asarray(Wq, np.float32)).astype(_FP8)
    wk_8 = (WS * np.asarray(Wk, np.float32)).astype(_FP8)
    wv_8 = (WS * np.asarray(Wv, np.float32)).astype(_FP8)
    wo_8 = (WS * np.asarray(Wo, np.float32)).astype(_FP8)
    bq_h = (WS * np.asarray(bq, np.float32)).reshape(MD, 1).astype(np.float32)
    bo_p = (np.asarray(bv, np.float64) @ np.asarray(Wo, np.float64)
            + np.asarray(bo, np.float64)).astype(np.float32).reshape(1, F)
    id_h = np.eye(P, dtype=np.float32).astype(_FP8)

    in_maps = []
    for c in range(NCORES):
        s = c * NQ
        xt_rot = np.concatenate([xt_full[:, s:], xt_full[:, :s]], axis=1)
        in_maps.append({
            "xt": np.ascontiguousarray(xt_rot).astype(_FP8),
            "wq": wq_8, "wk": wk_8, "wv": wv_8, "wo": wo_8,
            "bq": bq_h, "bo": bo_p, "id128": id_h,
        })
    return in_maps


def kernel(x, Wq, bq, Wk, bk, Wv, bv, Wo, bo):
    from concourse.bass_utils import run_bass_kernel_spmd

    in_maps = _make_in_maps(x, Wq, bq, Wk, bk, Wv, bv, Wo, bo)
    nc = _get_nc()
    res = run_bass_kernel_spmd(nc, in_maps, core_ids=list(range(NCORES)))
    return np.concatenate(
        [res.results[c]["out"] for c in range(NCORES)], axis=0)


def run_traced(x, Wq, bq, Wk, bk, Wv, bv, Wo, bo):
    """Like kernel() but with NTFF tracing; returns (output, exec_time_ns)."""
    from concourse.bass_utils import run_bass_kernel_spmd

    try:
        import ntff_shim
        ntff_shim.install()
    except ImportError:
        pass
    in_maps = _make_in_maps(x, Wq, bq, Wk, bk, Wv, bv, Wo, bo)
    nc = _get_nc()
    res = run_bass_kernel_spmd(nc, in_maps, core_ids=list(range(NCORES)),
                               trace=True)
    out = np.concatenate([res.results[c]["out"] for c in range(NCORES)], axis=0)
    return out, res.exec_time_ns


# revision 16
# speedup vs baseline: 1.0002x; 1.0002x over previous
"""Baseline kernel + fp8-DR output projection graft (kernel_bplus).

Identical to the original 469794ns baseline except:
- Output projection runs fp8 DoubleRow: ot tiles are fp8 (scaled 1/256),
  wo is fp8 (scaled 16x, OS = WS^2 so all factors cancel), 2 matmuls/qs
  instead of 4.
- id2 comes from a host-provided identity via DMA (faster startup than
  make_identity).
"""

import math
import sys

import numpy as np

for _p in ("/opt/trn_rl_repo", "/opt/pypackages"):
    if _p not in sys.path:
        sys.path.append(_p)

import ml_dtypes

N = 16384
F = 512
MD = 128
P = 128
NCORES = 8
NQ = N // NCORES
QB = 512
NQB = NQ // QB
JT = 128
NJT = N // JT
FK = F // P
CH = 512
NCH = N // CH
GK = 16
NG = NJT // GK
WS = 16.0
OS = 256.0
SCALE = 1.0 / math.sqrt(MD) / (WS * WS)

_BF16 = ml_dtypes.bfloat16
_FP8 = ml_dtypes.float8_e4m3fn


def _build():
    import concourse.bass as bass  # noqa: F401
    import concourse.tile as tile
    from concourse import bacc, mybir

    f32 = mybir.dt.float32
    bf16 = mybir.dt.bfloat16
    fp8 = mybir.dt.float8e4
    DR = mybir.MatmulPerfMode.DoubleRow
    AF = mybir.ActivationFunctionType
    ALU = mybir.AluOpType

    nc = bacc.Bacc("TRN2", target_bir_lowering=False, debug=False,
                   num_devices=NCORES)

    xt = nc.declare_dram_parameter("xt", [F, N], fp8, isOutput=False)
    wq = nc.declare_dram_parameter("wq", [F, MD], fp8, isOutput=False)
    wk = nc.declare_dram_parameter("wk", [F, MD], fp8, isOutput=False)
    wv = nc.declare_dram_parameter("wv", [F, F], fp8, isOutput=False)
    wo = nc.declare_dram_parameter("wo", [F, F], fp8, isOutput=False)
    bq = nc.declare_dram_parameter("bq", [MD, 1], f32, isOutput=False)
    bo = nc.declare_dram_parameter("bo", [1, F], f32, isOutput=False)
    id_in = nc.declare_dram_parameter("id128", [P, P], fp8, isOutput=False)
    out = nc.declare_dram_parameter("out", [NQ, F], f32, isOutput=True)

    with tile.TileContext(nc) as tc:
        with (
            tc.tile_pool(name="persist", bufs=1) as pp,
            tc.tile_pool(name="stream", bufs=4) as sp,
            tc.tile_pool(name="work", bufs=3) as wkp,
            tc.tile_pool(name="pssc", bufs=3, space="PSUM") as ps_sc,
            tc.tile_pool(name="pso", bufs=4, space="PSUM") as ps_o,
            tc.tile_pool(name="psesum", bufs=1, space="PSUM") as ps_es,
        ):
            # ---- persistent constants ------------------------------------
            id2 = pp.tile([P, 2, P], fp8, tag="id2")
            nc.sync.dma_start(out=id2[:, 0, :], in_=id_in[:])
            nc.sync.dma_start(out=id2[:, 1, :], in_=id_in[:])
            wq_a = pp.tile([P, FK, MD], fp8, tag="wqa")
            wk_a = pp.tile([P, FK, MD], fp8, tag="wka")
            wv_a = pp.tile([P, FK, F], fp8, tag="wva")
            wo2 = [pp.tile([P, 2, F], fp8, tag=f"wo{u}", name=f"wo{u}")
                   for u in range(2)]
            for k in range(FK):
                nc.sync.dma_start(out=wk_a[:, k, :], in_=wk[k * P:(k + 1) * P, :])
                nc.sync.dma_start(out=wv_a[:, k, :], in_=wv[k * P:(k + 1) * P, :])
            for u in range(2):
                for h in range(2):
                    r = 2 * u + h
                    nc.scalar.dma_start(out=wo2[u][:, h, :],
                                        in_=wo[r * P:(r + 1) * P, :])
                nc.scalar.dma_start(out=wq_a[:, 2 * u, :],
                                    in_=wq[2 * u * P:(2 * u + 1) * P, :])
                nc.scalar.dma_start(out=wq_a[:, 2 * u + 1, :],
                                    in_=wq[(2 * u + 1) * P:(2 * u + 2) * P, :])
            bq_t = pp.tile([MD, 1], f32, tag="bq")
            nc.scalar.dma_start(out=bq_t[:], in_=bq[:])
            bo_r = pp.tile([P, F], f32, tag="bor")
            nc.scalar.dma_start(out=bo_r[:], in_=bo[:].to_broadcast((P, F)))
            ones_f = pp.tile([P, 1], bf16, tag="ones")
            nc.vector.memset(ones_f[:], 1.0)

            # ---- persistent activations -----------------------------------
            ktg = [pp.tile([P, GK * JT], fp8, tag=f"ktg{g}", name=f"ktg{g}")
                   for g in range(NG)]
            vg = [pp.tile([P, GK * F], fp8, tag=f"vg{g}", name=f"vg{g}")
                  for g in range(NG)]
            qt = pp.tile([P, NQ], fp8, tag="qt")

            # ---- PE warmup during the initial DMA wait (HAM un-throttle) --
            warm_ps = ps_sc.tile([P, P], f32, tag="sc", name="warm_ps")
            for wi in range(20):
                nc.tensor.matmul(warm_ps[:], id2[:, 0, :], id2[:, 0, :],
                                 start=(wi == 0), stop=(wi == 19))
            warm_s = pp.tile([P, P], bf16, tag="warms")
            nc.scalar.copy(warm_s[:], warm_ps[:])

            # ---- prologue: project Q^T, K^T, V (fp8 DoubleRow) ------------
            for ch in range(NCH):
                xtc = sp.tile([P, FK, CH], fp8, tag="xtc")
                xt4 = xt.rearrange("(k p) n -> p k n", p=P)
                dma_eng = nc.gpsimd if ch % 2 == 0 else nc.sync
                if ch < 2:
                    for k in range(FK):
                        dma_eng.dma_start(
                            out=xtc[:, k, :],
                            in_=xt[k * P:(k + 1) * P, ch * CH:(ch + 1) * CH])
                else:
                    dma_eng.dma_start(
                        out=xtc[:], in_=xt4[:, :, ch * CH:(ch + 1) * CH])
                g, off = ch // 4, (ch % 4) * CH
                pk = ps_es.tile([P, CH], f32, tag="esum", name="pk")
                for h in range(2):
                    nc.tensor.matmul(pk[:], wk_a[:, 2 * h:2 * h + 2, :],
                                     xtc[:, 2 * h:2 * h + 2, :],
                                     start=(h == 0), stop=(h == 1), perf_mode=DR)
                nc.scalar.copy(ktg[g][:, off:off + CH], pk[:])
                if ch < NQ // CH:
                    pq = ps_sc.tile([P, CH], f32, tag="sc", name="pq")
                    for h in range(2):
                        nc.tensor.matmul(pq[:], wq_a[:, 2 * h:2 * h + 2, :],
                                         xtc[:, 2 * h:2 * h + 2, :],
                                         start=(h == 0), stop=(h == 1),
                                         perf_mode=DR)
                    nc.scalar.activation(qt[:, ch * CH:(ch + 1) * CH], pq[:],
                                         AF.Identity, bias=bq_t[:], scale=1.0)
                for js in range(CH // JT):
                    jt_g = ch * (CH // JT) + js
                    voff = (jt_g % GK) * F
                    pv = ps_o.tile([P, F], f32, tag="oacc", name="pv")
                    for h in range(2):
                        nc.tensor.matmul(
                            pv[:], xtc[:, 2 * h:2 * h + 2, js * JT:(js + 1) * JT],
                            wv_a[:, 2 * h:2 * h + 2, :],
                            start=(h == 0), stop=(h == 1), perf_mode=DR)
                    if jt_g % 2 == 0:
                        nc.vector.tensor_copy(vg[jt_g // GK][:, voff:voff + F],
                                              pv[:])
                    else:
                        nc.scalar.copy(vg[jt_g // GK][:, voff:voff + F], pv[:])

            # ---- attention: flat pipeline over all (q-block, key-pair) ----
            NP2 = NJT // 2
            SUMS_PAT = {1: "D", 2: "D", 3: "G", 4: "D", 5: "D", 6: "D",
                        7: "G"}

            def scores(gjt):
                qbb, jt_i = gjt // NJT, gjt % NJT
                g, r = jt_i // GK, jt_i % GK
                psc = ps_sc.tile([P, QB], f32, tag="sc", name="psc")
                nc.tensor.matmul(psc[:], ktg[g][:, r * JT:(r + 1) * JT],
                                 qt[:, qbb * QB:(qbb + 1) * QB],
                                 start=True, stop=True)
                return psc

            pending = {j: scores(j) for j in range(3)}
            state = {}
            deferred = [None]

            def epilogue(st):
                ot2 = st["ot2"]
                esb = wkp.tile([P, QB], bf16, tag="esb", bufs=2, name="esb")
                nc.vector.tensor_copy(esb[:], st["esum"][:])
                recip_p = wkp.tile([P, QB // P], f32, tag="recipp", bufs=2,
                                   name="recip_p")
                acc_d, acc_g = st["acc_d"], st["acc_g"]
                for qs in range(QB // P):
                    pt = ps_sc.tile([P, 1], f32, tag="sc", name="pt")
                    srcs = [acc_d[:, qs * P:(qs + 1) * P],
                            acc_d[:, QB + qs * P:QB + (qs + 1) * P],
                            acc_g[:, qs * P:(qs + 1) * P],
                            acc_g[:, QB + qs * P:QB + (qs + 1) * P],
                            esb[:, qs * P:(qs + 1) * P]]
                    for si, s in enumerate(srcs):
                        nc.tensor.matmul(pt[:], s, ones_f[:],
                                         start=(si == 0), stop=(si == 4))
                    nc.vector.reciprocal(recip_p[:, qs:qs + 1], pt[:])
                    pf = ps_sc.tile([P, F], f32, tag="sc", name="pf")
                    for u in range(2):
                        nc.tensor.matmul(
                            pf[:], ot2[u][:, :, qs * P:(qs + 1) * P], wo2[u][:],
                            start=(u == 0), stop=(u == 1), perf_mode=DR)
                    out_t = wkp.tile([P, F], f32, tag="outt", bufs=2, name="out_t")
                    nc.vector.scalar_tensor_tensor(
                        out_t[:], pf[:], recip_p[:, qs:qs + 1], bo_r[:],
                        ALU.mult, ALU.add)
                    row0 = st["qb"] * QB + qs * P
                    nc.sync.dma_start(out=out[row0:row0 + P, :], in_=out_t[:])

            for gp_i in range(NQB * NP2):
                qb, p_i = gp_i // NP2, gp_i % NP2
                if p_i == 0:
                    state = {
                        "qb": qb,
                        "po": [ps_o.tile([P, QB], f32, tag="oacc", name="oacc")
                               for _ in range(FK)],
                        "esum": ps_es.tile([P, QB], f32, tag="esum",
                                           name="esum"),
                        "acc_d": wkp.tile([P, 2 * QB], bf16, tag="accd", bufs=2,
                                          name="acc_d"),
                        "acc_g": wkp.tile([P, 2 * QB], bf16, tag="accg", bufs=2,
                                          name="acc_g"),
                        "seen": {"d": False, "g": False},
                    }
                jt0 = 2 * p_i
                g, r0 = jt0 // GK, jt0 % GK
                etp = wkp.tile([P, 2 * QB], fp8, tag="et", bufs=6)
                for h in range(2):
                    psc = pending.pop(qb * NJT + jt0 + h)
                    nc.scalar.activation(etp[:, h * QB:(h + 1) * QB], psc[:],
                                         AF.Exp, scale=SCALE)
                    nxt = qb * NJT + jt0 + h + 3
                    if nxt < NQB * NJT:
                        pending[nxt] = scores(nxt)
                et3 = etp.rearrange("p (h q) -> p h q", h=2)
                if p_i % 8 == 0:
                    nc.tensor.matmul(state["esum"][:], id2[:], et3,
                                     start=(p_i == 0), stop=(p_i == NP2 - 8),
                                     perf_mode=DR)
                else:
                    kind = SUMS_PAT[p_i % 8]
                    eng, acc, key = ((nc.vector, state["acc_d"], "d")
                                     if kind == "D"
                                     else (nc.gpsimd, state["acc_g"], "g"))
                    if not state["seen"][key]:
                        eng.tensor_copy(acc[:], etp[:])
                        state["seen"][key] = True
                    else:
                        eng.tensor_tensor(acc[:], acc[:], etp[:], ALU.add)
                vg4 = vg[g].rearrange("p (t h f) -> p t h f", h=2, f=F)
                for ft in range(FK):
                    nc.tensor.matmul(
                        state["po"][ft][:],
                        vg4[:, r0 // 2, :, ft * P:(ft + 1) * P],
                        et3, start=(p_i == 0), stop=(p_i == NP2 - 1),
                        perf_mode=DR)
                if p_i == 1 and deferred[0] is not None:
                    epilogue(deferred[0])
                    deferred[0] = None
                if p_i == NP2 - 1:
                    ot2 = [wkp.tile([P, 2, QB], fp8, tag=f"ot{u}", bufs=2,
                                    name=f"ot2_{u}") for u in range(2)]
                    for u in range(2):
                        nc.scalar.activation(
                            ot2[u][:, 0, :], state["po"][2 * u][:],
                            AF.Copy, scale=1.0 / OS)
                        nc.vector.tensor_scalar_mul(
                            ot2[u][:, 1, :], state["po"][2 * u + 1][:],
                            1.0 / OS)
                    state["ot2"] = ot2
                    deferred[0] = state
            epilogue(deferred[0])

    nc.compile()
    return nc


_CACHED = {}


def _get_nc():
    if "nc" not in _CACHED:
        _CACHED["nc"] = _build()
    return _CACHED["nc"]


def _make_in_maps(x, Wq, bq, Wk, bk, Wv, bv, Wo, bo):
    x = np.asarray(x, dtype=np.float32)
    xt_full = np.ascontiguousarray(x.T)                     # [F, N] f32
    wq_8 = (WS * np.asarray(Wq, np.float32)).astype(_FP8)
    wk_8 = (WS * np.asarray(Wk, np.float32)).astype(_FP8)
    wv_8 = (WS * np.asarray(Wv, np.float32)).astype(_FP8)
    wo_8 = (WS * np.asarray(Wo, np.float32)).astype(_FP8)
    bq_h = (WS * np.asarray(bq, np.float32)).reshape(MD, 1).astype(np.float32)
    bo_p = (np.asarray(bv, np.float64) @ np.asarray(Wo, np.float64)
            + np.asarray(bo, np.float64)).astype(np.float32).reshape(1, F)
    id_h = np.eye(P, dtype=np.float32).astype(_FP8)

    in_maps = []
    for c in range(NCORES):
        s = c * NQ
        xt_rot = np.concatenate([xt_full[:, s:], xt_full[:, :s]], axis=1)
        in_maps.append({
            "xt": np.ascontiguousarray(xt_rot).astype(_FP8),
            "wq": wq_8, "wk": wk_8, "wv": wv_8, "wo": wo_8,
            "bq": bq_h, "bo": bo_p, "id128": id_h,
        })
    return in_maps


def kernel(x, Wq, bq, Wk, bk, Wv, bv, Wo, bo):
    from concourse.bass_utils import run_bass_kernel_spmd

    in_maps = _make_in_maps(x, Wq, bq, Wk, bk, Wv, bv, Wo, bo)
    nc = _get_nc()
    res = run_bass_kernel_spmd(nc, in_maps, core_ids=list(range(NCORES)))
    return np.concatenate(
        [res.results[c]["out"] for c in range(NCORES)], axis=0)


def run_traced(x, Wq, bq, Wk, bk, Wv, bv, Wo, bo):
    """Like kernel() but with NTFF tracing; returns (output, exec_time_ns)."""
    from concourse.bass_utils import run_bass_kernel_spmd

    try:
        import ntff_shim
        ntff_shim.install()
    except ImportError:
        pass
    in_maps = _make_in_maps(x, Wq, bq, Wk, bk, Wv, bv, Wo, bo)
    nc = _get_nc()
    res = run_bass_kernel_spmd(nc, in_maps, core_ids=list(range(NCORES)),
                               trace=True)
    out = np.concatenate([res.results[c]["out"] for c in range(NCORES)], axis=0)
    return out, res.exec_time_ns
